# revision 13
# baseline (speedup 1.0000x reference)
"""Multi-head attention (B=4, T=2048, D=1024, H=16) on 8 Trainium2 cores.

Sharding: core c handles (batch b = c//2, head-group g = c%2) — 8 heads,
512 output features. No inter-core communication.

Host-side: rows of K/V masked out by mask_attn and rows of Q masked out by
mask_out are compacted away (their probabilities / outputs are exactly zero
in the reference), then padded to a multiple of 128. Activations and
weight slices are pre-transposed so every device matmul contracts over the
partition dim, and converted to bf16 (PSUM accumulation is fp32).

Device per core: project K/Q into transposed [feature, token] layout and V
into natural [token, feature] layout (biases added via K=1 ones-matmuls);
scores^T = K_h @ Q_h^T per head pair, packed into disjoint PE row groups;
one ScalarE instruction applies scale + key-padding bias + exp per 2-head
PSUM tile; PV accumulates [V_h | 1]^T @ probs^T giving the output and the
softmax denominator (ones column). Projections of head-pair j+1 are
emission-interleaved into pair j's ACT-bound attention loop to fill PE
idle slots. Host divides by the denominator and scatters rows.
"""

import itertools
import os
import sys

sys.path.insert(0, "/opt/trn_rl_repo")

import numpy as np
import ml_dtypes
from contextlib import ExitStack

import concourse.bacc as bacc
import concourse.tile as tile
from concourse import mybir
from concourse.bass_utils import run_bass_kernel_spmd

F32 = mybir.dt.float32
BF16 = mybir.dt.bfloat16

HID = 1024
FO = 512          # projection features per core = 8 heads * 64
HPC = 8           # heads per core
NFI = HID // 128  # contraction chunks
N_CORES = 8


def _tiles(total, w):
    out = []
    o = 0
    while o < total:
        tw = min(w, total - o)
        out.append((o, tw))
        o += tw
    return out


def _build(TQ, TK):
    NTK = TK // 128
    TQT = _tiles(TQ, 512)

    nc = bacc.Bacc("TRN2", target_bir_lowering=False, debug=False)

    qT_d = nc.declare_dram_parameter("qT", [128, NFI * TQ], BF16, isOutput=False)
    kT_d = nc.declare_dram_parameter("kT", [128, NFI * TK], BF16, isOutput=False)
    vT_d = nc.declare_dram_parameter("vT", [128, NFI * TK], BF16, isOutput=False)
    wqT_d = nc.declare_dram_parameter("wqT", [128, NFI * FO], BF16, isOutput=False)
    wkT_d = nc.declare_dram_parameter("wkT", [128, NFI * FO], BF16, isOutput=False)
    wvT_d = nc.declare_dram_parameter("wvT", [128, NFI * FO], BF16, isOutput=False)
    bq_d = nc.declare_dram_parameter("bq", [1, FO], BF16, isOutput=False)
    bk_d = nc.declare_dram_parameter("bk", [1, FO], BF16, isOutput=False)
    bv_d = nc.declare_dram_parameter("bv", [1, FO], BF16, isOutput=False)
    biask_d = nc.declare_dram_parameter("biask", [128, NTK], F32, isOutput=False)
    ones1_d = nc.declare_dram_parameter("ones1", [1, 512], BF16, isOutput=False)
    onesv_d = nc.declare_dram_parameter("onesv", [128, NTK * HPC], BF16, isOutput=False)
    out_d = nc.declare_dram_parameter("out", [65, HPC, TQ], F32, isOutput=True)

    Exp = mybir.ActivationFunctionType.Exp

    with tile.TileContext(nc) as tc, ExitStack() as ctx:
        res = ctx.enter_context(tc.tile_pool(name="res", bufs=1))
        qhT = res.tile([128, 4, TQ], BF16)        # [fo%128, pair, t]
        khT = res.tile([128, 4, TK], BF16)
        vh = res.tile([128, NTK, HPC, 65], BF16)  # [t%128, t//128, head, dh+1]
        ones = res.tile([1, 512], BF16)
        biask_sb = res.tile([128, NTK], F32)
        bq_sb = res.tile([1, FO], BF16)
        bk_sb = res.tile([1, FO], BF16)
        bv_sb = res.tile([1, FO], BF16)
        kT_sb = res.tile([128, NFI * TK], BF16)
        qT_sb = res.tile([128, NFI * TQ], BF16)
        vT_sb = res.tile([128, NFI * TK], BF16)
        wq_sb = res.tile([128, NFI, FO], BF16)
        wk_sb = res.tile([128, NFI, FO], BF16)
        wv_sb = res.tile([128, NFI, FO], BF16)

        def tview(sb, t0, tw, T):
            # blocks of 512 tokens packed [c, t] along the free dim
            b0 = (t0 // 512) * 512
            bw = min(512, T - b0)
            blk = sb[:, b0 * NFI:(b0 + bw) * NFI].rearrange(
                "p (c t) -> p c t", c=NFI
            )
            return blk[:, :, t0 - b0:t0 - b0 + tw]

        # Constants + V-path via gpsimd SWDGE; K-path on the sync ring;
        # vT/qT on the scalar HWDGE ring (idle until attention starts).
        nc.gpsimd.dma_start(biask_sb[:], biask_d[:])
        nc.gpsimd.dma_start(bk_sb[:], bk_d[:])
        nc.gpsimd.dma_start(bq_sb[:], bq_d[:])
        nc.gpsimd.dma_start(bv_sb[:], bv_d[:])
        nc.gpsimd.dma_start(ones[:], ones1_d[:])
        nc.gpsimd.dma_start(vh[:, :, :, 64:65], onesv_d[:])
        def _stream(sb, dd, t0, tw):
            nc.sync.dma_start(
                sb[:, t0 * NFI:(t0 + tw) * NFI],
                dd[:, t0 * NFI:(t0 + tw) * NFI],
            )

        kt_tiles = _tiles(TK, 512)
        qt_tiles = _tiles(TQ, 512)
        nc.sync.dma_start(wk_sb[:].rearrange("p c n -> p (c n)"), wkT_d[:])
        _stream(kT_sb, kT_d, *kt_tiles[0])
        nc.sync.dma_start(wq_sb[:].rearrange("p c n -> p (c n)"), wqT_d[:])
        _stream(qT_sb, qT_d, *qt_tiles[0])
        nc.sync.dma_start(wv_sb[:].rearrange("p c n -> p (c n)"), wvT_d[:])
        _stream(vT_sb, vT_d, *kt_tiles[0])
        for tt in kt_tiles[1:]:
            _stream(kT_sb, kT_d, *tt)
        for tt in kt_tiles[1:]:
            _stream(vT_sb, vT_d, *tt)
        for tt in qt_tiles[1:]:
            _stream(qT_sb, qT_d, *tt)

        ppj = ctx.enter_context(tc.tile_pool(name="ppj", bufs=2, space="PSUM"))
        scps = ctx.enter_context(tc.tile_pool(name="scps", bufs=2, space="PSUM"))
        otps = ctx.enter_context(tc.tile_pool(name="otps", bufs=1, space="PSUM"))
        probs_pool = ctx.enter_context(tc.tile_pool(name="probs", bufs=4))
        park_pool = ctx.enter_context(tc.tile_pool(name="park", bufs=4))

        def gen_kq_proj(jf, src_sb, w_sb, b_sb, dst, nT):
            """Projection of feature tile jf (one head pair), [fo, t] layout."""
            for (t0, tw) in _tiles(nT, 512):
                ps = ppj.tile([128, 512], F32, name="pjps")
                tv = tview(src_sb, t0, tw, nT)
                for c in range(NFI):
                    nc.tensor.matmul(
                        ps[:, :tw],
                        w_sb[:, c, jf * 128:(jf + 1) * 128],
                        tv[:, c, :],
                        start=(c == 0), stop=False,
                    )
                nc.tensor.matmul(
                    ps[:, :tw],
                    b_sb[0:1, jf * 128:(jf + 1) * 128],
                    ones[0:1, :tw],
                    start=False, stop=True,
                )
                nc.vector.tensor_copy(dst[:, jf, t0:t0 + tw], ps[:, :tw])
                yield

        def gen_v_proj(half):
            """V projection for heads 4*half .. 4*half+3, natural layout."""
            f0 = half * 256
            for it in range(NTK):
                ps = ppj.tile([128, 512], F32, name="pjps")
                tvv = tview(vT_sb, it * 128, 128, TK)
                for c in range(NFI):
                    nc.tensor.matmul(
                        ps[:, :256],
                        tvv[:, c, :],
                        wv_sb[:, c, f0:f0 + 256],
                        start=(c == 0), stop=False,
                    )
                nc.tensor.matmul(
                    ps[:, :256], ones[0:1, 0:128], bv_sb[0:1, f0:f0 + 256],
                    start=False, stop=True,
                )
                nc.vector.tensor_copy(
                    vh[:, it, 4 * half:4 * half + 4, 0:64],
                    ps[:, :256].rearrange("p (h d) -> p h d", h=4),
                )
                yield

        def drain(g):
            for _ in g:
                pass

        def pump(g, n):
            for _ in range(n):
                if next(g, None) is None:
                    return

        # Upfront: only the first tile of K/Q/V projections for pair 0;
        # the remainder streams inside the attention slot loop.
        g_k0 = gen_kq_proj(0, kT_sb, wk_sb, bk_sb, khT, TK)
        g_q0 = gen_kq_proj(0, qT_sb, wq_sb, bq_sb, qhT, TQ)
        g_v0 = gen_v_proj(0)
        pump(g_k0, 1)
        drain(g_q0)
        pump(g_v0, 1)

        # Flattened attention pipeline over (pair, tq-tile, tk) slots.
        # PV of slot i is emitted after scores of slot i+1, so the in-order
        # PE never parks behind a PV that waits on the ACT output.
        slots = [
            (j, ti, t0, tw, it)
            for j in range(4)
            for ti, (t0, tw) in enumerate(TQT)
            for it in range(NTK)
        ]
        pair_gens = {}
        pump_rate = {}
        for j in range(3):
            gens = [
                gen_kq_proj(j + 1, kT_sb, wk_sb, bk_sb, khT, TK),
                gen_kq_proj(j + 1, qT_sb, wq_sb, bq_sb, qhT, TQ),
            ]
            if j == 0:
                gens = [g_k0, g_v0] + gens
            if j == 1:
                gens.insert(0, gen_v_proj(1))
            pair_gens[j] = itertools.chain(*gens)
            pump_rate[j] = 2 if j <= 1 else 1
        pair_gens[3] = iter(())
        pump_rate[3] = 0

        otiles = {}
        pending = []

        cur_o = {}

        def emit_pv(slot):
            j, ti, t0, tw, it = slot
            pr = otiles.pop((j, ti, it))
            o0, o1 = cur_o[(j, ti)]
            nc.tensor.matmul(
                o0[:, :tw], vh[:, it, 2 * j, :], pr[:, 0, :tw],
                start=(it == 0), stop=(it == NTK - 1),
            )
            nc.tensor.matmul(
                o1[:, :tw], vh[:, it, 2 * j + 1, :], pr[:, 1, :tw],
                start=(it == 0), stop=(it == NTK - 1),
            )
            if it == NTK - 1:
                del cur_o[(j, ti)]
                pk0 = park_pool.tile([65, 512], F32, name="pk")
                nc.vector.tensor_copy(pk0[:, :tw], o0[:, :tw])
                nc.sync.dma_start(out_d[:, 2 * j, t0:t0 + tw], pk0[:, :tw])
                pk1 = park_pool.tile([65, 512], F32, name="pk")
                nc.vector.tensor_copy(pk1[:, :tw], o1[:, :tw])
                nc.sync.dma_start(out_d[:, 2 * j + 1, t0:t0 + tw], pk1[:, :tw])

        cur_pair = 0
        for slot in slots:
            j, ti, t0, tw, it = slot
            if j != cur_pair:
                # everything pair j needs must be emitted before its scores
                drain(pair_gens[cur_pair])
                cur_pair = j
            sp = scps.tile([128, 2, 512], F32, name="sc")
            nc.tensor.matmul(
                sp[:, 0, :tw],
                khT[0:64, j, it * 128:(it + 1) * 128],
                qhT[0:64, j, t0:t0 + tw],
                start=True, stop=True,
            )
            nc.tensor.matmul(
                sp[:, 1, :tw],
                khT[64:128, j, it * 128:(it + 1) * 128],
                qhT[64:128, j, t0:t0 + tw],
                start=True, stop=True,
            )
            pump(pair_gens[j], pump_rate[j])
            pr = probs_pool.tile([128, 2, 512], BF16, name="pr")
            nc.scalar.activation(
                pr[:, :, :tw], sp[:, :, :tw], Exp,
                bias=biask_sb[:, it:it + 1], scale=0.125,
            )
            if it == 0:
                o0 = otps.tile([65, 512], F32, name="ot0")
                o1 = otps.tile([65, 512], F32, name="ot1")
                cur_o[(j, ti)] = (o0, o1)
            otiles[(j, ti, it)] = pr
            pending.append(slot)
            if len(pending) > 2:
                emit_pv(pending.pop(0))
        for s in pending:
            emit_pv(s)
        drain(pair_gens[3])

    nc.finalize()
    return nc


def _swz_act(x):
    """[T, HID] -> [128, NFI*T] packed as 512-token blocks of [NFI, tw]."""
    T = x.shape[0]
    xt = np.ascontiguousarray(x.T).reshape(NFI, 128, T).transpose(1, 0, 2)
    blocks = [
        xt[:, :, t0:t0 + tw].reshape(128, -1) for (t0, tw) in _tiles(T, 512)
    ]
    return np.concatenate(blocks, axis=1).astype(ml_dtypes.bfloat16)


def _swz_w(w):
    """[FO, HID] -> [128, NFI*FO]: w.T reshaped to [128, NFI, FO] flat."""
    wt = np.ascontiguousarray(w.T).reshape(NFI, 128, FO).transpose(1, 0, 2)
    return np.ascontiguousarray(wt.reshape(128, NFI * FO)).astype(ml_dtypes.bfloat16)


def kernel(q, k, v, Wq, bq, Wk, bk, Wv, bv, mask_attn, mask_out):
    q = np.asarray(q, np.float32)
    k = np.asarray(k, np.float32)
    v = np.asarray(v, np.float32)
    Wq = np.asarray(Wq, np.float32)
    Wk = np.asarray(Wk, np.float32)
    Wv = np.asarray(Wv, np.float32)
    bq = np.asarray(bq, np.float32)
    bk = np.asarray(bk, np.float32)
    bv = np.asarray(bv, np.float32)
    mask_attn = np.asarray(mask_attn)
    mask_out = np.asarray(mask_out)

    B, T, _ = q.shape
    idxk = [np.flatnonzero(mask_attn[b]) for b in range(B)]
    idxq = [np.flatnonzero(mask_out[b]) for b in range(B)]
    TK = max(128, -(-max(len(i) for i in idxk) // 128) * 128)
    TQ = max(128, -(-max(len(i) for i in idxq) // 128) * 128)
    NTK = TK // 128

    nc = _build(TQ, TK)

    in_maps = []
    for c in range(N_CORES):
        b, g = c // 2, c % 2
        sl = slice(g * FO, (g + 1) * FO)
        nk, nq = len(idxk[b]), len(idxq[b])
        qc = np.zeros((TQ, HID), np.float32)
        qc[:nq] = q[b][idxq[b]]
        kc = np.zeros((TK, HID), np.float32)
        kc[:nk] = k[b][idxk[b]]
        vc = np.zeros((TK, HID), np.float32)
        vc[:nk] = v[b][idxk[b]]
        biask = np.full(TK, -30000.0, np.float32)
        biask[:nk] = 0.0
        in_maps.append({
            "qT": _swz_act(qc),
            "kT": _swz_act(kc),
            "vT": _swz_act(vc),
            "wqT": _swz_w(Wq[sl]),
            "wkT": _swz_w(Wk[sl]),
            "wvT": _swz_w(Wv[sl]),
            "bq": bq[sl].reshape(1, FO).astype(ml_dtypes.bfloat16),
            "bk": bk[sl].reshape(1, FO).astype(ml_dtypes.bfloat16),
            "bv": bv[sl].reshape(1, FO).astype(ml_dtypes.bfloat16),
            "biask": np.ascontiguousarray(biask.reshape(NTK, 128).T),
            "ones1": np.ones((1, 512), ml_dtypes.bfloat16),
            "onesv": np.ones((128, NTK * HPC), ml_dtypes.bfloat16),
        })

    trace_dir = os.environ.get("KERNEL_TRACE_DIR")
    if trace_dir:
        res = run_bass_kernel_spmd(
            nc, in_maps, list(range(N_CORES)), trace=True, tmpdir=trace_dir
        )
        print(f"HW exec time: {res.exec_time_ns} ns")
    else:
        res = run_bass_kernel_spmd(nc, in_maps, list(range(N_CORES)))

    out_full = np.zeros((B, T, HID), np.float32)
    for c in range(N_CORES):
        b, g = c // 2, c % 2
        nq = len(idxq[b])
        u = res.results[c]["out"]  # [65, HPC, TQ]
        o = u[:64, :, :nq] / u[64:65, :, :nq]
        o = o.transpose(2, 1, 0).reshape(nq, FO)
        out_full[b, idxq[b], g * FO:(g + 1) * FO] = o
    return out_full


# revision 14
# speedup vs baseline: 1.0387x; 1.0387x over previous
"""Multi-head attention (B=4, T=2048, D=1024, H=16) on 8 Trainium2 cores.

Sharding: core c handles (batch b = c//2, head-group g = c%2) — 8 heads,
512 output features. No inter-core communication.

Host-side: rows of K/V masked out by mask_attn and rows of Q masked out by
mask_out are compacted away (their probabilities / outputs are exactly zero
in the reference), then padded to a multiple of 128. Activations and
weight slices are pre-transposed so every device matmul contracts over the
partition dim, and converted to bf16 (PSUM accumulation is fp32).

Device per core: project K/Q into transposed [feature, token] layout and V
into natural [token, feature] layout (biases added via K=1 ones-matmuls);
scores^T = K_h @ Q_h^T per head pair, packed into disjoint PE row groups;
one ScalarE instruction applies scale + key-padding bias + exp per 2-head
PSUM tile; PV accumulates [V_h | 1]^T @ probs^T giving the output and the
softmax denominator (ones column). Projections of head-pair j+1 are
emission-interleaved into pair j's ACT-bound attention loop to fill PE
idle slots. Host divides by the denominator and scatters rows.
"""

import itertools
import os
import sys

sys.path.insert(0, "/opt/trn_rl_repo")

import numpy as np
import ml_dtypes
from contextlib import ExitStack

import concourse.bacc as bacc
import concourse.tile as tile
from concourse import mybir
from concourse.bass_utils import run_bass_kernel_spmd

F32 = mybir.dt.float32
BF16 = mybir.dt.bfloat16

HID = 1024
FO = 512          # projection features per core = 8 heads * 64
HPC = 8           # heads per core
NFI = HID // 128  # contraction chunks
N_CORES = 8


def _tiles(total, w):
    out = []
    o = 0
    while o < total:
        tw = min(w, total - o)
        out.append((o, tw))
        o += tw
    return out


def _build(TQ, TK):
    NTK = TK // 128
    TQT = _tiles(TQ, 512)

    nc = bacc.Bacc("TRN2", target_bir_lowering=False, debug=False)

    qT_d = nc.declare_dram_parameter("qT", [128, NFI * TQ], BF16, isOutput=False)
    kT_d = nc.declare_dram_parameter("kT", [128, NFI * TK], BF16, isOutput=False)
    vT_d = nc.declare_dram_parameter("vT", [128, NFI * TK], BF16, isOutput=False)
    wqT_d = nc.declare_dram_parameter("wqT", [128, NFI * FO], BF16, isOutput=False)
    wkT_d = nc.declare_dram_parameter("wkT", [128, NFI * FO], BF16, isOutput=False)
    wvT_d = nc.declare_dram_parameter("wvT", [128, NFI * FO], BF16, isOutput=False)
    bq_d = nc.declare_dram_parameter("bq", [1, FO], BF16, isOutput=False)
    bk_d = nc.declare_dram_parameter("bk", [1, FO], BF16, isOutput=False)
    bv_d = nc.declare_dram_parameter("bv", [1, FO], BF16, isOutput=False)
    biask_d = nc.declare_dram_parameter("biask", [128, NTK], F32, isOutput=False)
    ones1_d = nc.declare_dram_parameter("ones1", [1, 512], BF16, isOutput=False)
    onesv_d = nc.declare_dram_parameter("onesv", [128, NTK * HPC], BF16, isOutput=False)
    out_d = nc.declare_dram_parameter("out", [65, HPC, TQ], F32, isOutput=True)

    Exp = mybir.ActivationFunctionType.Exp

    with tile.TileContext(nc) as tc, ExitStack() as ctx:
        res = ctx.enter_context(tc.tile_pool(name="res", bufs=1))
        qhT = res.tile([128, 4, TQ], BF16)        # [fo%128, pair, t]
        khT = res.tile([128, 4, TK], BF16)
        vh = res.tile([128, NTK, HPC, 65], BF16)  # [t%128, t//128, head, dh+1]
        ones = res.tile([1, 512], BF16)
        biask_sb = res.tile([128, NTK], F32)
        bq_sb = res.tile([1, FO], BF16)
        bk_sb = res.tile([1, FO], BF16)
        bv_sb = res.tile([1, FO], BF16)
        kT_sb = res.tile([128, NFI * TK], BF16)
        qT_sb = res.tile([128, NFI * TQ], BF16)
        vT_sb = res.tile([128, NFI * TK], BF16)
        wq_sb = res.tile([128, NFI, FO], BF16)
        wk_sb = res.tile([128, NFI, FO], BF16)
        wv_sb = res.tile([128, NFI, FO], BF16)

        def tview(sb, t0, tw, T):
            # blocks of 512 tokens packed [c, t] along the free dim
            b0 = (t0 // 512) * 512
            bw = min(512, T - b0)
            blk = sb[:, b0 * NFI:(b0 + bw) * NFI].rearrange(
                "p (c t) -> p c t", c=NFI
            )
            return blk[:, :, t0 - b0:t0 - b0 + tw]

        # Constants + V-path via gpsimd SWDGE; K-path on the sync ring;
        # vT/qT on the scalar HWDGE ring (idle until attention starts).
        nc.gpsimd.dma_start(biask_sb[:], biask_d[:])
        nc.gpsimd.dma_start(bk_sb[:], bk_d[:])
        nc.gpsimd.dma_start(bq_sb[:], bq_d[:])
        nc.gpsimd.dma_start(bv_sb[:], bv_d[:])
        nc.gpsimd.dma_start(ones[:], ones1_d[:])
        nc.gpsimd.dma_start(vh[:, :, :, 64:65], onesv_d[:])
        def _stream(sb, dd, t0, tw):
            nc.sync.dma_start(
                sb[:, t0 * NFI:(t0 + tw) * NFI],
                dd[:, t0 * NFI:(t0 + tw) * NFI],
            )

        kt_tiles = _tiles(TK, 512)
        qt_tiles = _tiles(TQ, 512)
        nc.sync.dma_start(wk_sb[:].rearrange("p c n -> p (c n)"), wkT_d[:])
        _stream(kT_sb, kT_d, *kt_tiles[0])
        nc.sync.dma_start(wv_sb[:].rearrange("p c n -> p (c n)"), wvT_d[:])
        _stream(vT_sb, vT_d, *kt_tiles[0])
        nc.sync.dma_start(wq_sb[:].rearrange("p c n -> p (c n)"), wqT_d[:])
        _stream(qT_sb, qT_d, *qt_tiles[0])
        for i in range(1, max(len(kt_tiles), len(qt_tiles))):
            if i < len(kt_tiles):
                _stream(kT_sb, kT_d, *kt_tiles[i])
                _stream(vT_sb, vT_d, *kt_tiles[i])
            if i < len(qt_tiles):
                _stream(qT_sb, qT_d, *qt_tiles[i])

        ppj = ctx.enter_context(tc.tile_pool(name="ppj", bufs=2, space="PSUM"))
        scps = ctx.enter_context(tc.tile_pool(name="scps", bufs=2, space="PSUM"))
        otps = ctx.enter_context(tc.tile_pool(name="otps", bufs=1, space="PSUM"))
        probs_pool = ctx.enter_context(tc.tile_pool(name="probs", bufs=4))
        park_pool = ctx.enter_context(tc.tile_pool(name="park", bufs=4))

        def gen_kq_proj(jf, src_sb, w_sb, b_sb, dst, nT):
            """Projection of feature tile jf (one head pair), [fo, t] layout."""
            for (t0, tw) in _tiles(nT, 512):
                ps = ppj.tile([128, 512], F32, name="pjps")
                tv = tview(src_sb, t0, tw, nT)
                for c in range(NFI):
                    nc.tensor.matmul(
                        ps[:, :tw],
                        w_sb[:, c, jf * 128:(jf + 1) * 128],
                        tv[:, c, :],
                        start=(c == 0), stop=False,
                    )
                nc.tensor.matmul(
                    ps[:, :tw],
                    b_sb[0:1, jf * 128:(jf + 1) * 128],
                    ones[0:1, :tw],
                    start=False, stop=True,
                )
                nc.vector.tensor_copy(dst[:, jf, t0:t0 + tw], ps[:, :tw])
                yield

        def gen_v_proj(half):
            """V projection for heads 4*half .. 4*half+3, natural layout."""
            f0 = half * 256
            for it in range(NTK):
                ps = ppj.tile([128, 512], F32, name="pjps")
                tvv = tview(vT_sb, it * 128, 128, TK)
                for c in range(NFI):
                    nc.tensor.matmul(
                        ps[:, :256],
                        tvv[:, c, :],
                        wv_sb[:, c, f0:f0 + 256],
                        start=(c == 0), stop=False,
                    )
                nc.tensor.matmul(
                    ps[:, :256], ones[0:1, 0:128], bv_sb[0:1, f0:f0 + 256],
                    start=False, stop=True,
                )
                nc.vector.tensor_copy(
                    vh[:, it, 4 * half:4 * half + 4, 0:64],
                    ps[:, :256].rearrange("p (h d) -> p h d", h=4),
                )
                yield

        def drain(g):
            for _ in g:
                pass

        def pump(g, n):
            for _ in range(n):
                if next(g, None) is None:
                    return

        # Upfront: only the first tile of K/Q/V projections for pair 0;
        # the remainder streams inside the attention slot loop.
        g_k0 = gen_kq_proj(0, kT_sb, wk_sb, bk_sb, khT, TK)
        g_q0 = gen_kq_proj(0, qT_sb, wq_sb, bq_sb, qhT, TQ)
        g_v0 = gen_v_proj(0)
        pump(g_k0, 1)
        pump(g_q0, 1)
        pump(g_v0, 1)

        # Flattened attention pipeline over (pair, tq-tile, tk) slots.
        # PV of slot i is emitted after scores of slot i+1, so the in-order
        # PE never parks behind a PV that waits on the ACT output.
        slots = [
            (j, ti, t0, tw, it)
            for j in range(4)
            for ti, (t0, tw) in enumerate(TQT)
            for it in range(NTK)
        ]
        class PairPump:
            """One tile from the primary (V) gen + one from the rest, RR."""

            def __init__(self, primary, rest):
                self.primary = primary
                self.rest = list(rest)
                self.i = 0

            def step(self):
                if self.primary is not None:
                    if next(self.primary, _DONE) is _DONE:
                        self.primary = None
                for _ in range(len(self.rest)):
                    g = self.rest[self.i % len(self.rest)]
                    self.i += 1
                    if next(g, _DONE) is not _DONE:
                        return
                    self.rest.remove(g)
                    if not self.rest:
                        return

            def drain(self):
                if self.primary is not None:
                    for _ in self.primary:
                        pass
                    self.primary = None
                for g in self.rest:
                    for _ in g:
                        pass
                self.rest = []

        _DONE = object()
        pair_gens = {
            0: PairPump(g_v0, [
                g_k0, g_q0,
                gen_kq_proj(1, kT_sb, wk_sb, bk_sb, khT, TK),
                gen_kq_proj(1, qT_sb, wq_sb, bq_sb, qhT, TQ),
            ]),
            1: PairPump(gen_v_proj(1), [
                gen_kq_proj(2, kT_sb, wk_sb, bk_sb, khT, TK),
                gen_kq_proj(2, qT_sb, wq_sb, bq_sb, qhT, TQ),
            ]),
            2: PairPump(None, [
                gen_kq_proj(3, kT_sb, wk_sb, bk_sb, khT, TK),
                gen_kq_proj(3, qT_sb, wq_sb, bq_sb, qhT, TQ),
            ]),
            3: PairPump(None, []),
        }

        otiles = {}
        pending = []

        cur_o = {}

        def emit_pv(slot):
            j, ti, t0, tw, it = slot
            pr = otiles.pop((j, ti, it))
            o0, o1 = cur_o[(j, ti)]
            nc.tensor.matmul(
                o0[:, :tw], vh[:, it, 2 * j, :], pr[:, 0, :tw],
                start=(it == 0), stop=(it == NTK - 1),
            )
            nc.tensor.matmul(
                o1[:, :tw], vh[:, it, 2 * j + 1, :], pr[:, 1, :tw],
                start=(it == 0), stop=(it == NTK - 1),
            )
            if it == NTK - 1:
                del cur_o[(j, ti)]
                pk0 = park_pool.tile([65, 512], F32, name="pk")
                nc.vector.tensor_copy(pk0[:, :tw], o0[:, :tw])
                nc.sync.dma_start(out_d[:, 2 * j, t0:t0 + tw], pk0[:, :tw])
                pk1 = park_pool.tile([65, 512], F32, name="pk")
                nc.vector.tensor_copy(pk1[:, :tw], o1[:, :tw])
                nc.sync.dma_start(out_d[:, 2 * j + 1, t0:t0 + tw], pk1[:, :tw])

        cur_pair = 0
        for slot in slots:
            j, ti, t0, tw, it = slot
            if j != cur_pair:
                # everything pair j needs must be emitted before its scores
                pair_gens[cur_pair].drain()
                cur_pair = j
            sp = scps.tile([128, 2, 512], F32, name="sc")
            nc.tensor.matmul(
                sp[:, 0, :tw],
                khT[0:64, j, it * 128:(it + 1) * 128],
                qhT[0:64, j, t0:t0 + tw],
                start=True, stop=True,
            )
            nc.tensor.matmul(
                sp[:, 1, :tw],
                khT[64:128, j, it * 128:(it + 1) * 128],
                qhT[64:128, j, t0:t0 + tw],
                start=True, stop=True,
            )
            pair_gens[j].step()
            pr = probs_pool.tile([128, 2, 512], BF16, name="pr")
            nc.scalar.activation(
                pr[:, :, :tw], sp[:, :, :tw], Exp,
                bias=biask_sb[:, it:it + 1], scale=0.125,
            )
            if it == 0:
                o0 = otps.tile([65, 512], F32, name="ot0")
                o1 = otps.tile([65, 512], F32, name="ot1")
                cur_o[(j, ti)] = (o0, o1)
            otiles[(j, ti, it)] = pr
            pending.append(slot)
            if len(pending) > 2:
                emit_pv(pending.pop(0))
        for s in pending:
            emit_pv(s)
        pair_gens[3].drain()

    nc.finalize()
    return nc


def _swz_act(x):
    """[T, HID] -> [128, NFI*T] packed as 512-token blocks of [NFI, tw]."""
    T = x.shape[0]
    xt = np.ascontiguousarray(x.T).reshape(NFI, 128, T).transpose(1, 0, 2)
    blocks = [
        xt[:, :, t0:t0 + tw].reshape(128, -1) for (t0, tw) in _tiles(T, 512)
    ]
    return np.concatenate(blocks, axis=1).astype(ml_dtypes.bfloat16)


def _swz_w(w):
    """[FO, HID] -> [128, NFI*FO]: w.T reshaped to [128, NFI, FO] flat."""
    wt = np.ascontiguousarray(w.T).reshape(NFI, 128, FO).transpose(1, 0, 2)
    return np.ascontiguousarray(wt.reshape(128, NFI * FO)).astype(ml_dtypes.bfloat16)


def kernel(q, k, v, Wq, bq, Wk, bk, Wv, bv, mask_attn, mask_out):
    q = np.asarray(q, np.float32)
    k = np.asarray(k, np.float32)
    v = np.asarray(v, np.float32)
    Wq = np.asarray(Wq, np.float32)
    Wk = np.asarray(Wk, np.float32)
    Wv = np.asarray(Wv, np.float32)
    bq = np.asarray(bq, np.float32)
    bk = np.asarray(bk, np.float32)
    bv = np.asarray(bv, np.float32)
    mask_attn = np.asarray(mask_attn)
    mask_out = np.asarray(mask_out)

    B, T, _ = q.shape
    idxk = [np.flatnonzero(mask_attn[b]) for b in range(B)]
    idxq = [np.flatnonzero(mask_out[b]) for b in range(B)]
    TK = max(128, -(-max(len(i) for i in idxk) // 128) * 128)
    TQ = max(128, -(-max(len(i) for i in idxq) // 128) * 128)
    NTK = TK // 128

    nc = _build(TQ, TK)

    in_maps = []
    for c in range(N_CORES):
        b, g = c // 2, c % 2
        sl = slice(g * FO, (g + 1) * FO)
        nk, nq = len(idxk[b]), len(idxq[b])
        qc = np.zeros((TQ, HID), np.float32)
        qc[:nq] = q[b][idxq[b]]
        kc = np.zeros((TK, HID), np.float32)
        kc[:nk] = k[b][idxk[b]]
        vc = np.zeros((TK, HID), np.float32)
        vc[:nk] = v[b][idxk[b]]
        biask = np.full(TK, -30000.0, np.float32)
        biask[:nk] = 0.0
        in_maps.append({
            "qT": _swz_act(qc),
            "kT": _swz_act(kc),
            "vT": _swz_act(vc),
            "wqT": _swz_w(Wq[sl]),
            "wkT": _swz_w(Wk[sl]),
            "wvT": _swz_w(Wv[sl]),
            "bq": bq[sl].reshape(1, FO).astype(ml_dtypes.bfloat16),
            "bk": bk[sl].reshape(1, FO).astype(ml_dtypes.bfloat16),
            "bv": bv[sl].reshape(1, FO).astype(ml_dtypes.bfloat16),
            "biask": np.ascontiguousarray(biask.reshape(NTK, 128).T),
            "ones1": np.ones((1, 512), ml_dtypes.bfloat16),
            "onesv": np.ones((128, NTK * HPC), ml_dtypes.bfloat16),
        })

    trace_dir = os.environ.get("KERNEL_TRACE_DIR")
    if trace_dir:
        res = run_bass_kernel_spmd(
            nc, in_maps, list(range(N_CORES)), trace=True, tmpdir=trace_dir
        )
        print(f"HW exec time: {res.exec_time_ns} ns")
    else:
        res = run_bass_kernel_spmd(nc, in_maps, list(range(N_CORES)))

    out_full = np.zeros((B, T, HID), np.float32)
    for c in range(N_CORES):
        b, g = c // 2, c % 2
        nq = len(idxq[b])
        u = res.results[c]["out"]  # [65, HPC, TQ]
        o = u[:64, :, :nq] / u[64:65, :, :nq]
        o = o.transpose(2, 1, 0).reshape(nq, FO)
        out_full[b, idxq[b], g * FO:(g + 1) * FO] = o
    return out_full


# revision 15
# speedup vs baseline: 1.0583x; 1.0188x over previous
"""Multi-head attention (B=4, T=2048, D=1024, H=16) on 8 Trainium2 cores.

Sharding: core c handles (batch b = c//2, head-group g = c%2) — 8 heads,
512 output features. No inter-core communication.

Host-side: rows of K/V masked out by mask_attn and rows of Q masked out by
mask_out are compacted away (their probabilities / outputs are exactly zero
in the reference), then padded to a multiple of 128. Activations and
weight slices are pre-transposed so every device matmul contracts over the
partition dim, and converted to bf16 (PSUM accumulation is fp32).

Device per core: project K/Q into transposed [feature, token] layout and V
into natural [token, feature] layout (biases added via K=1 ones-matmuls);
scores^T = K_h @ Q_h^T per head pair, packed into disjoint PE row groups;
one ScalarE instruction applies scale + key-padding bias + exp per 2-head
PSUM tile; PV accumulates [V_h | 1]^T @ probs^T giving the output and the
softmax denominator (ones column). Projections of head-pair j+1 are
emission-interleaved into pair j's ACT-bound attention loop to fill PE
idle slots. Host divides by the denominator and scatters rows.
"""

import itertools
import os
import sys

sys.path.insert(0, "/opt/trn_rl_repo")

import numpy as np
import ml_dtypes
from contextlib import ExitStack

import concourse.bacc as bacc
import concourse.tile as tile
from concourse import mybir
from concourse.bass_utils import run_bass_kernel_spmd

F32 = mybir.dt.float32
BF16 = mybir.dt.bfloat16

HID = 1024
FO = 512          # projection features per core = 8 heads * 64
HPC = 8           # heads per core
NFI = HID // 128  # contraction chunks
N_CORES = 8


def _tiles(total, w):
    out = []
    o = 0
    while o < total:
        tw = min(w, total - o)
        out.append((o, tw))
        o += tw
    return out


def _build(TQ, TK):
    NTK = TK // 128
    TQT = _tiles(TQ, 512)

    nc = bacc.Bacc("TRN2", target_bir_lowering=False, debug=False)

    qT_d = nc.declare_dram_parameter("qT", [128, NFI * TQ], BF16, isOutput=False)
    kT_d = nc.declare_dram_parameter("kT", [128, NFI * TK], BF16, isOutput=False)
    vT_d = nc.declare_dram_parameter("vT", [128, NFI * TK], BF16, isOutput=False)
    wqT_d = nc.declare_dram_parameter("wqT", [128, NFI * FO], BF16, isOutput=False)
    wkT_d = nc.declare_dram_parameter("wkT", [128, NFI * FO], BF16, isOutput=False)
    wvT_d = nc.declare_dram_parameter("wvT", [128, NFI * FO], BF16, isOutput=False)
    bq_d = nc.declare_dram_parameter("bq", [128, 4], F32, isOutput=False)
    bk_d = nc.declare_dram_parameter("bk", [128, 4], F32, isOutput=False)
    bv_d = nc.declare_dram_parameter("bv", [1, FO], BF16, isOutput=False)
    biask_d = nc.declare_dram_parameter("biask", [128, NTK], F32, isOutput=False)
    ones1_d = nc.declare_dram_parameter("ones1", [1, 512], BF16, isOutput=False)
    onesv_d = nc.declare_dram_parameter("onesv", [128, NTK * HPC], BF16, isOutput=False)
    out_d = nc.declare_dram_parameter("out", [65, HPC, TQ], F32, isOutput=True)

    Exp = mybir.ActivationFunctionType.Exp

    with tile.TileContext(nc) as tc, ExitStack() as ctx:
        res = ctx.enter_context(tc.tile_pool(name="res", bufs=1))
        qhT = res.tile([128, 4, TQ], BF16)        # [fo%128, pair, t]
        khT = res.tile([128, 4, TK], BF16)
        vh = res.tile([128, NTK, HPC, 65], BF16)  # [t%128, t//128, head, dh+1]
        ones = res.tile([1, 512], BF16)
        biask_sb = res.tile([128, NTK], F32)
        bq_sb = res.tile([128, 4], F32)
        bk_sb = res.tile([128, 4], F32)
        bv_sb = res.tile([1, FO], BF16)
        kT_sb = res.tile([128, NFI * TK], BF16)
        qT_sb = res.tile([128, NFI * TQ], BF16)
        vT_sb = res.tile([128, NFI * TK], BF16)
        wq_sb = res.tile([128, NFI, FO], BF16)
        wk_sb = res.tile([128, NFI, FO], BF16)
        wv_sb = res.tile([128, NFI, FO], BF16)

        def tview(sb, t0, tw, T):
            # blocks of 512 tokens packed [c, t] along the free dim
            b0 = (t0 // 512) * 512
            bw = min(512, T - b0)
            blk = sb[:, b0 * NFI:(b0 + bw) * NFI].rearrange(
                "p (c t) -> p c t", c=NFI
            )
            return blk[:, :, t0 - b0:t0 - b0 + tw]

        # Constants + V-path via gpsimd SWDGE; K-path on the sync ring;
        # vT/qT on the scalar HWDGE ring (idle until attention starts).
        nc.gpsimd.dma_start(biask_sb[:], biask_d[:])
        nc.gpsimd.dma_start(bk_sb[:], bk_d[:])
        nc.gpsimd.dma_start(bq_sb[:], bq_d[:])
        nc.gpsimd.dma_start(bv_sb[:], bv_d[:])
        nc.gpsimd.dma_start(ones[:], ones1_d[:])
        nc.gpsimd.dma_start(vh[:, :, :, 64:65], onesv_d[:])
        def _stream(sb, dd, t0, tw):
            nc.sync.dma_start(
                sb[:, t0 * NFI:(t0 + tw) * NFI],
                dd[:, t0 * NFI:(t0 + tw) * NFI],
            )

        kt_tiles = _tiles(TK, 512)
        qt_tiles = _tiles(TQ, 512)
        nc.sync.dma_start(wk_sb[:].rearrange("p c n -> p (c n)"), wkT_d[:])
        _stream(kT_sb, kT_d, *kt_tiles[0])
        nc.sync.dma_start(wq_sb[:].rearrange("p c n -> p (c n)"), wqT_d[:])
        _stream(qT_sb, qT_d, *qt_tiles[0])
        nc.sync.dma_start(wv_sb[:].rearrange("p c n -> p (c n)"), wvT_d[:])
        _stream(vT_sb, vT_d, *kt_tiles[0])
        for i in range(1, max(len(kt_tiles), len(qt_tiles))):
            if i < len(kt_tiles):
                _stream(kT_sb, kT_d, *kt_tiles[i])
                _stream(vT_sb, vT_d, *kt_tiles[i])
            if i < len(qt_tiles):
                _stream(qT_sb, qT_d, *qt_tiles[i])

        ppj = ctx.enter_context(tc.tile_pool(name="ppj", bufs=2, space="PSUM"))
        scps = ctx.enter_context(tc.tile_pool(name="scps", bufs=2, space="PSUM"))
        otps = ctx.enter_context(tc.tile_pool(name="otps", bufs=1, space="PSUM"))
        probs_pool = ctx.enter_context(tc.tile_pool(name="probs", bufs=4))
        park_pool = ctx.enter_context(tc.tile_pool(name="park", bufs=4))

        def gen_kq_proj(jf, src_sb, w_sb, b_sb, dst, nT):
            """Projection of feature tile jf (one head pair), [fo, t] layout."""
            for (t0, tw) in _tiles(nT, 512):
                ps = ppj.tile([128, 512], F32, name="pjps")
                tv = tview(src_sb, t0, tw, nT)
                for c in range(NFI):
                    nc.tensor.matmul(
                        ps[:, :tw],
                        w_sb[:, c, jf * 128:(jf + 1) * 128],
                        tv[:, c, :],
                        start=(c == 0), stop=(c == NFI - 1),
                    )
                nc.vector.tensor_scalar_add(
                    dst[:, jf, t0:t0 + tw], ps[:, :tw], b_sb[:, jf:jf + 1]
                )
                yield

        def gen_v_proj(half):
            """V projection for heads 4*half .. 4*half+3, natural layout."""
            f0 = half * 256
            for it in range(NTK):
                ps = ppj.tile([128, 512], F32, name="pjps")
                tvv = tview(vT_sb, it * 128, 128, TK)
                for c in range(NFI):
                    nc.tensor.matmul(
                        ps[:, :256],
                        tvv[:, c, :],
                        wv_sb[:, c, f0:f0 + 256],
                        start=(c == 0), stop=False,
                    )
                nc.tensor.matmul(
                    ps[:, :256], ones[0:1, 0:128], bv_sb[0:1, f0:f0 + 256],
                    start=False, stop=True,
                )
                nc.vector.tensor_copy(
                    vh[:, it, 4 * half:4 * half + 4, 0:64],
                    ps[:, :256].rearrange("p (h d) -> p h d", h=4),
                )
                yield

        def drain(g):
            for _ in g:
                pass

        def pump(g, n):
            for _ in range(n):
                if next(g, None) is None:
                    return

        # Upfront: only the first tile of K/Q/V projections for pair 0;
        # the remainder streams inside the attention slot loop.
        g_k0 = gen_kq_proj(0, kT_sb, wk_sb, bk_sb, khT, TK)
        g_q0 = gen_kq_proj(0, qT_sb, wq_sb, bq_sb, qhT, TQ)
        g_v0 = gen_v_proj(0)
        pump(g_k0, 1)
        pump(g_q0, 1)
        pump(g_v0, 1)

        # Flattened attention pipeline over (pair, tq-tile, tk) slots.
        # PV of slot i is emitted after scores of slot i+1, so the in-order
        # PE never parks behind a PV that waits on the ACT output.
        slots = [
            (j, ti, t0, tw, it)
            for j in range(4)
            for ti, (t0, tw) in enumerate(TQT)
            for it in range(NTK)
        ]
        class PairPump:
            """One tile from the primary (V) gen + one from the rest, RR."""

            def __init__(self, primary, rest):
                self.primary = primary
                self.rest = list(rest)
                self.i = 0

            def step(self):
                if self.primary is not None:
                    if next(self.primary, _DONE) is _DONE:
                        self.primary = None
                for _ in range(len(self.rest)):
                    g = self.rest[self.i % len(self.rest)]
                    self.i += 1
                    if next(g, _DONE) is not _DONE:
                        return
                    self.rest.remove(g)
                    if not self.rest:
                        return

            def drain(self):
                if self.primary is not None:
                    for _ in self.primary:
                        pass
                    self.primary = None
                for g in self.rest:
                    for _ in g:
                        pass
                self.rest = []

        _DONE = object()
        pair_gens = {
            0: PairPump(g_v0, [
                g_k0, g_q0,
                gen_kq_proj(1, kT_sb, wk_sb, bk_sb, khT, TK),
                gen_kq_proj(1, qT_sb, wq_sb, bq_sb, qhT, TQ),
            ]),
            1: PairPump(gen_v_proj(1), [
                gen_kq_proj(2, kT_sb, wk_sb, bk_sb, khT, TK),
                gen_kq_proj(2, qT_sb, wq_sb, bq_sb, qhT, TQ),
            ]),
            2: PairPump(None, [
                gen_kq_proj(3, kT_sb, wk_sb, bk_sb, khT, TK),
                gen_kq_proj(3, qT_sb, wq_sb, bq_sb, qhT, TQ),
            ]),
            3: PairPump(None, []),
        }

        otiles = {}
        pending = []

        cur_o = {}

        def emit_pv(slot):
            j, ti, t0, tw, it = slot
            pr = otiles.pop((j, ti, it))
            o0, o1 = cur_o[(j, ti)]
            nc.tensor.matmul(
                o0[:, :tw], vh[:, it, 2 * j, :], pr[:, 0, :tw],
                start=(it == 0), stop=(it == NTK - 1),
            )
            nc.tensor.matmul(
                o1[:, :tw], vh[:, it, 2 * j + 1, :], pr[:, 1, :tw],
                start=(it == 0), stop=(it == NTK - 1),
            )
            if it == NTK - 1:
                del cur_o[(j, ti)]
                pk0 = park_pool.tile([65, 512], F32, name="pk")
                nc.vector.tensor_copy(pk0[:, :tw], o0[:, :tw])
                nc.sync.dma_start(out_d[:, 2 * j, t0:t0 + tw], pk0[:, :tw])
                pk1 = park_pool.tile([65, 512], F32, name="pk")
                nc.vector.tensor_copy(pk1[:, :tw], o1[:, :tw])
                nc.sync.dma_start(out_d[:, 2 * j + 1, t0:t0 + tw], pk1[:, :tw])

        cur_pair = 0
        for slot in slots:
            j, ti, t0, tw, it = slot
            if j != cur_pair:
                # everything pair j needs must be emitted before its scores
                pair_gens[cur_pair].drain()
                cur_pair = j
            sp = scps.tile([128, 2, 512], F32, name="sc")
            nc.tensor.matmul(
                sp[:, 0, :tw],
                khT[0:64, j, it * 128:(it + 1) * 128],
                qhT[0:64, j, t0:t0 + tw],
                start=True, stop=True,
            )
            nc.tensor.matmul(
                sp[:, 1, :tw],
                khT[64:128, j, it * 128:(it + 1) * 128],
                qhT[64:128, j, t0:t0 + tw],
                start=True, stop=True,
            )
            pair_gens[j].step()
            pr = probs_pool.tile([128, 2, 512], BF16, name="pr")
            nc.scalar.activation(
                pr[:, :, :tw], sp[:, :, :tw], Exp,
                bias=biask_sb[:, it:it + 1], scale=0.125,
            )
            if it == 0:
                o0 = otps.tile([65, 512], F32, name="ot0")
                o1 = otps.tile([65, 512], F32, name="ot1")
                cur_o[(j, ti)] = (o0, o1)
            otiles[(j, ti, it)] = pr
            pending.append(slot)
            if len(pending) > 2:
                emit_pv(pending.pop(0))
        for s in pending:
            emit_pv(s)
        pair_gens[3].drain()

    nc.finalize()
    return nc


def _swz_act(x):
    """[T, HID] -> [128, NFI*T] packed as 512-token blocks of [NFI, tw]."""
    T = x.shape[0]
    xt = np.ascontiguousarray(x.T).reshape(NFI, 128, T).transpose(1, 0, 2)
    blocks = [
        xt[:, :, t0:t0 + tw].reshape(128, -1) for (t0, tw) in _tiles(T, 512)
    ]
    return np.concatenate(blocks, axis=1).astype(ml_dtypes.bfloat16)


def _swz_w(w):
    """[FO, HID] -> [128, NFI*FO]: w.T reshaped to [128, NFI, FO] flat."""
    wt = np.ascontiguousarray(w.T).reshape(NFI, 128, FO).transpose(1, 0, 2)
    return np.ascontiguousarray(wt.reshape(128, NFI * FO)).astype(ml_dtypes.bfloat16)


def kernel(q, k, v, Wq, bq, Wk, bk, Wv, bv, mask_attn, mask_out):
    q = np.asarray(q, np.float32)
    k = np.asarray(k, np.float32)
    v = np.asarray(v, np.float32)
    Wq = np.asarray(Wq, np.float32)
    Wk = np.asarray(Wk, np.float32)
    Wv = np.asarray(Wv, np.float32)
    bq = np.asarray(bq, np.float32)
    bk = np.asarray(bk, np.float32)
    bv = np.asarray(bv, np.float32)
    mask_attn = np.asarray(mask_attn)
    mask_out = np.asarray(mask_out)

    B, T, _ = q.shape
    idxk = [np.flatnonzero(mask_attn[b]) for b in range(B)]
    idxq = [np.flatnonzero(mask_out[b]) for b in range(B)]
    TK = max(128, -(-max(len(i) for i in idxk) // 128) * 128)
    TQ = max(128, -(-max(len(i) for i in idxq) // 128) * 128)
    NTK = TK // 128

    nc = _build(TQ, TK)

    in_maps = []
    for c in range(N_CORES):
        b, g = c // 2, c % 2
        sl = slice(g * FO, (g + 1) * FO)
        nk, nq = len(idxk[b]), len(idxq[b])
        qc = np.zeros((TQ, HID), np.float32)
        qc[:nq] = q[b][idxq[b]]
        kc = np.zeros((TK, HID), np.float32)
        kc[:nk] = k[b][idxk[b]]
        vc = np.zeros((TK, HID), np.float32)
        vc[:nk] = v[b][idxk[b]]
        biask = np.full(TK, -30000.0, np.float32)
        biask[:nk] = 0.0
        in_maps.append({
            "qT": _swz_act(qc),
            "kT": _swz_act(kc),
            "vT": _swz_act(vc),
            "wqT": _swz_w(Wq[sl]),
            "wkT": _swz_w(Wk[sl]),
            "wvT": _swz_w(Wv[sl]),
            "bq": np.ascontiguousarray(bq[sl].reshape(4, 128).T),
            "bk": np.ascontiguousarray(bk[sl].reshape(4, 128).T),
            "bv": bv[sl].reshape(1, FO).astype(ml_dtypes.bfloat16),
            "biask": np.ascontiguousarray(biask.reshape(NTK, 128).T),
            "ones1": np.ones((1, 512), ml_dtypes.bfloat16),
            "onesv": np.ones((128, NTK * HPC), ml_dtypes.bfloat16),
        })

    trace_dir = os.environ.get("KERNEL_TRACE_DIR")
    if trace_dir:
        res = run_bass_kernel_spmd(
            nc, in_maps, list(range(N_CORES)), trace=True, tmpdir=trace_dir
        )
        print(f"HW exec time: {res.exec_time_ns} ns")
    else:
        res = run_bass_kernel_spmd(nc, in_maps, list(range(N_CORES)))

    out_full = np.zeros((B, T, HID), np.float32)
    for c in range(N_CORES):
        b, g = c // 2, c % 2
        nq = len(idxq[b])
        u = res.results[c]["out"]  # [65, HPC, TQ]
        o = u[:64, :, :nq] / u[64:65, :, :nq]
        o = o.transpose(2, 1, 0).reshape(nq, FO)
        out_full[b, idxq[b], g * FO:(g + 1) * FO] = o
    return out_full


# revision 16
# speedup vs baseline: 1.1177x; 1.0561x over previous
"""Multi-head attention (B=4, T=2048, D=1024, H=16) on 8 Trainium2 cores.

Sharding: core c handles (batch b = c//2, head-group g = c%2) — 8 heads,
512 output features. No inter-core communication.

Host-side: rows of K/V masked out by mask_attn and rows of Q masked out by
mask_out are compacted away (their probabilities / outputs are exactly zero
in the reference), then padded to a multiple of 128. Activations and
weight slices are pre-transposed so every device matmul contracts over the
partition dim, and converted to bf16 (PSUM accumulation is fp32).

Device per core: project K/Q into transposed [feature, token] layout and V
into natural [token, feature] layout (biases added via K=1 ones-matmuls);
scores^T = K_h @ Q_h^T per head pair, packed into disjoint PE row groups;
one ScalarE instruction applies scale + key-padding bias + exp per 2-head
PSUM tile; PV accumulates [V_h | 1]^T @ probs^T giving the output and the
softmax denominator (ones column). Projections of head-pair j+1 are
emission-interleaved into pair j's ACT-bound attention loop to fill PE
idle slots. Host divides by the denominator and scatters rows.
"""

import itertools
import os
import sys

sys.path.insert(0, "/opt/trn_rl_repo")

import numpy as np
import ml_dtypes
from contextlib import ExitStack

import concourse.bacc as bacc
import concourse.tile as tile
from concourse import mybir
from concourse.bass_utils import run_bass_kernel_spmd

F32 = mybir.dt.float32
BF16 = mybir.dt.bfloat16

HID = 1024
FO = 512          # projection features per core = 8 heads * 64
HPC = 8           # heads per core
NFI = HID // 128  # contraction chunks
N_CORES = 8


def _tiles(total, w):
    out = []
    o = 0
    while o < total:
        tw = min(w, total - o)
        out.append((o, tw))
        o += tw
    return out


def _blocks(total, first):
    """First block small (fast DMA landing), then 512-wide blocks."""
    if total <= first:
        return [(0, total)]
    return [(0, first)] + [(first + o, w) for (o, w) in _tiles(total - first, 512)]


def _build(TQ, TK):
    NTK = TK // 128
    TQT = _blocks(TQ, 256)
    KQB_K = _blocks(TK, 256)
    VB = _blocks(TK, 128)

    nc = bacc.Bacc("TRN2", target_bir_lowering=False, debug=False)

    qT_d = nc.declare_dram_parameter("qT", [128, NFI * TQ], BF16, isOutput=False)
    kT_d = nc.declare_dram_parameter("kT", [128, NFI * TK], BF16, isOutput=False)
    vT_d = nc.declare_dram_parameter("vT", [128, NFI * TK], BF16, isOutput=False)
    wqT_d = nc.declare_dram_parameter("wqT", [128, NFI * FO], BF16, isOutput=False)
    wkT_d = nc.declare_dram_parameter("wkT", [128, NFI * FO], BF16, isOutput=False)
    wvT_d = nc.declare_dram_parameter("wvT", [128, NFI * FO], BF16, isOutput=False)
    bq_d = nc.declare_dram_parameter("bq", [128, 4], F32, isOutput=False)
    bk_d = nc.declare_dram_parameter("bk", [128, 4], F32, isOutput=False)
    bv_d = nc.declare_dram_parameter("bv", [1, FO], BF16, isOutput=False)
    biask_d = nc.declare_dram_parameter("biask", [128, NTK], F32, isOutput=False)
    ones1_d = nc.declare_dram_parameter("ones1", [1, 512], BF16, isOutput=False)
    onesv_d = nc.declare_dram_parameter("onesv", [128, NTK * HPC], BF16, isOutput=False)
    out_d = nc.declare_dram_parameter("out", [65, HPC, TQ], F32, isOutput=True)

    Exp = mybir.ActivationFunctionType.Exp

    with tile.TileContext(nc) as tc, ExitStack() as ctx:
        res = ctx.enter_context(tc.tile_pool(name="res", bufs=1))
        qhT = res.tile([128, 4, TQ], BF16)        # [fo%128, pair, t]
        khT = res.tile([128, 4, TK], BF16)
        vh = res.tile([128, NTK, HPC, 65], BF16)  # [t%128, t//128, head, dh+1]
        ones = res.tile([1, 512], BF16)
        biask_sb = res.tile([128, NTK], F32)
        bq_sb = res.tile([128, 4], F32)
        bk_sb = res.tile([128, 4], F32)
        bv_sb = res.tile([1, FO], BF16)
        kT_sb = res.tile([128, NFI * TK], BF16)
        qT_sb = res.tile([128, NFI * TQ], BF16)
        vT_sb = res.tile([128, NFI * TK], BF16)
        wq_sb = res.tile([128, NFI, FO], BF16)
        wk_sb = res.tile([128, NFI, FO], BF16)
        wv_sb = res.tile([128, NFI, FO], BF16)

        def tview(sb, t0, tw, blocks):
            # token blocks packed [c, t] along the free dim
            for (b0, bw) in blocks:
                if b0 <= t0 and t0 + tw <= b0 + bw:
                    blk = sb[:, b0 * NFI:(b0 + bw) * NFI].rearrange(
                        "p (c t) -> p c t", c=NFI
                    )
                    return blk[:, :, t0 - b0:t0 - b0 + tw]
            raise AssertionError((t0, tw, blocks))

        # Constants + V-path via gpsimd SWDGE; K-path on the sync ring;
        # vT/qT on the scalar HWDGE ring (idle until attention starts).
        nc.gpsimd.dma_start(biask_sb[:], biask_d[:])
        nc.gpsimd.dma_start(bk_sb[:], bk_d[:])
        nc.gpsimd.dma_start(bq_sb[:], bq_d[:])
        nc.gpsimd.dma_start(bv_sb[:], bv_d[:])
        nc.gpsimd.dma_start(ones[:], ones1_d[:])
        nc.gpsimd.dma_start(vh[:, :, :, 64:65], onesv_d[:])
        def _stream(sb, dd, t0, tw):
            nc.sync.dma_start(
                sb[:, t0 * NFI:(t0 + tw) * NFI],
                dd[:, t0 * NFI:(t0 + tw) * NFI],
            )

        nc.sync.dma_start(wk_sb[:].rearrange("p c n -> p (c n)"), wkT_d[:])
        _stream(kT_sb, kT_d, *KQB_K[0])
        nc.sync.dma_start(wq_sb[:].rearrange("p c n -> p (c n)"), wqT_d[:])
        _stream(qT_sb, qT_d, *TQT[0])
        nc.sync.dma_start(wv_sb[:].rearrange("p c n -> p (c n)"), wvT_d[:])
        _stream(vT_sb, vT_d, *VB[0])
        for i in range(1, max(len(KQB_K), len(TQT), len(VB))):
            if i < len(KQB_K):
                _stream(kT_sb, kT_d, *KQB_K[i])
            if i < len(VB):
                _stream(vT_sb, vT_d, *VB[i])
            if i < len(TQT):
                _stream(qT_sb, qT_d, *TQT[i])

        ppj = ctx.enter_context(tc.tile_pool(name="ppj", bufs=2, space="PSUM"))
        scps = ctx.enter_context(tc.tile_pool(name="scps", bufs=2, space="PSUM"))
        otps = ctx.enter_context(tc.tile_pool(name="otps", bufs=1, space="PSUM"))
        probs_pool = ctx.enter_context(tc.tile_pool(name="probs", bufs=4))
        park_pool = ctx.enter_context(tc.tile_pool(name="park", bufs=4))

        def gen_kq_proj(jf, src_sb, w_sb, b_sb, dst, blocks):
            """Projection of feature tile jf (one head pair), [fo, t] layout."""
            for (t0, tw) in blocks:
                ps = ppj.tile([128, 512], F32, name="pjps")
                tv = tview(src_sb, t0, tw, blocks)
                for c in range(NFI):
                    nc.tensor.matmul(
                        ps[:, :tw],
                        w_sb[:, c, jf * 128:(jf + 1) * 128],
                        tv[:, c, :],
                        start=(c == 0), stop=(c == NFI - 1),
                    )
                nc.vector.tensor_scalar_add(
                    dst[:, jf, t0:t0 + tw], ps[:, :tw], b_sb[:, jf:jf + 1]
                )
                yield

        def gen_v_proj(half):
            """V projection for heads 4*half .. 4*half+3, natural layout."""
            f0 = half * 256
            for it in range(NTK):
                ps = ppj.tile([128, 512], F32, name="pjps")
                tvv = tview(vT_sb, it * 128, 128, VB)
                for c in range(NFI):
                    nc.tensor.matmul(
                        ps[:, :256],
                        tvv[:, c, :],
                        wv_sb[:, c, f0:f0 + 256],
                        start=(c == 0), stop=False,
                    )
                nc.tensor.matmul(
                    ps[:, :256], ones[0:1, 0:128], bv_sb[0:1, f0:f0 + 256],
                    start=False, stop=True,
                )
                nc.vector.tensor_copy(
                    vh[:, it, 4 * half:4 * half + 4, 0:64],
                    ps[:, :256].rearrange("p (h d) -> p h d", h=4),
                )
                yield

        def drain(g):
            for _ in g:
                pass

        def pump(g, n):
            for _ in range(n):
                if next(g, None) is None:
                    return

        # Upfront: only the first tile of K/Q/V projections for pair 0;
        # the remainder streams inside the attention slot loop.
        g_k0 = gen_kq_proj(0, kT_sb, wk_sb, bk_sb, khT, KQB_K)
        g_q0 = gen_kq_proj(0, qT_sb, wq_sb, bq_sb, qhT, TQT)
        g_v0 = gen_v_proj(0)
        pump(g_k0, 1)
        pump(g_q0, 1)
        pump(g_v0, 1)

        # Flattened attention pipeline over (pair, tq-tile, tk) slots.
        # PV of slot i is emitted after scores of slot i+1, so the in-order
        # PE never parks behind a PV that waits on the ACT output.
        slots = [
            (j, ti, t0, tw, it)
            for j in range(4)
            for ti, (t0, tw) in enumerate(TQT)
            for it in range(NTK)
        ]
        class PairPump:
            """One tile from the primary (V) gen + one from the rest, RR."""

            def __init__(self, primary, rest):
                self.primary = primary
                self.rest = list(rest)
                self.i = 0

            def step(self):
                if self.primary is not None:
                    if next(self.primary, _DONE) is _DONE:
                        self.primary = None
                for _ in range(len(self.rest)):
                    g = self.rest[self.i % len(self.rest)]
                    self.i += 1
                    if next(g, _DONE) is not _DONE:
                        return
                    self.rest.remove(g)
                    if not self.rest:
                        return

            def drain(self):
                if self.primary is not None:
                    for _ in self.primary:
                        pass
                    self.primary = None
                for g in self.rest:
                    for _ in g:
                        pass
                self.rest = []

        _DONE = object()
        pair_gens = {
            0: PairPump(g_v0, [
                g_k0, g_q0,
                gen_kq_proj(1, kT_sb, wk_sb, bk_sb, khT, KQB_K),
                gen_kq_proj(1, qT_sb, wq_sb, bq_sb, qhT, TQT),
            ]),
            1: PairPump(gen_v_proj(1), [
                gen_kq_proj(2, kT_sb, wk_sb, bk_sb, khT, KQB_K),
                gen_kq_proj(2, qT_sb, wq_sb, bq_sb, qhT, TQT),
            ]),
            2: PairPump(None, [
                gen_kq_proj(3, kT_sb, wk_sb, bk_sb, khT, KQB_K),
                gen_kq_proj(3, qT_sb, wq_sb, bq_sb, qhT, TQT),
            ]),
            3: PairPump(None, []),
        }

        otiles = {}
        pending = []

        cur_o = {}

        def emit_pv(slot):
            j, ti, t0, tw, it = slot
            pr = otiles.pop((j, ti, it))
            o0, o1 = cur_o[(j, ti)]
            nc.tensor.matmul(
                o0[:, :tw], vh[:, it, 2 * j, :], pr[:, 0, :tw],
                start=(it == 0), stop=(it == NTK - 1),
            )
            nc.tensor.matmul(
                o1[:, :tw], vh[:, it, 2 * j + 1, :], pr[:, 1, :tw],
                start=(it == 0), stop=(it == NTK - 1),
            )
            if it == NTK - 1:
                del cur_o[(j, ti)]
                pk0 = park_pool.tile([65, 512], F32, name="pk")
                nc.vector.tensor_copy(pk0[:, :tw], o0[:, :tw])
                nc.sync.dma_start(out_d[:, 2 * j, t0:t0 + tw], pk0[:, :tw])
                pk1 = park_pool.tile([65, 512], F32, name="pk")
                nc.vector.tensor_copy(pk1[:, :tw], o1[:, :tw])
                nc.sync.dma_start(out_d[:, 2 * j + 1, t0:t0 + tw], pk1[:, :tw])

        cur_pair = 0
        for slot in slots:
            j, ti, t0, tw, it = slot
            if j != cur_pair:
                # everything pair j needs must be emitted before its scores
                pair_gens[cur_pair].drain()
                cur_pair = j
            sp = scps.tile([128, 2, 512], F32, name="sc")
            nc.tensor.matmul(
                sp[:, 0, :tw],
                khT[0:64, j, it * 128:(it + 1) * 128],
                qhT[0:64, j, t0:t0 + tw],
                start=True, stop=True,
            )
            nc.tensor.matmul(
                sp[:, 1, :tw],
                khT[64:128, j, it * 128:(it + 1) * 128],
                qhT[64:128, j, t0:t0 + tw],
                start=True, stop=True,
            )
            pair_gens[j].step()
            pr = probs_pool.tile([128, 2, 512], BF16, name="pr")
            nc.scalar.activation(
                pr[:, :, :tw], sp[:, :, :tw], Exp,
                bias=biask_sb[:, it:it + 1], scale=0.125,
            )
            if it == 0:
                o0 = otps.tile([65, 512], F32, name="ot0")
                o1 = otps.tile([65, 512], F32, name="ot1")
                cur_o[(j, ti)] = (o0, o1)
            otiles[(j, ti, it)] = pr
            pending.append(slot)
            if len(pending) > 2:
                emit_pv(pending.pop(0))
        for s in pending:
            emit_pv(s)
        pair_gens[3].drain()

    nc.finalize()
    return nc


def _swz_act(x, first):
    """[T, HID] -> [128, NFI*T] packed as token blocks of [NFI, tw]."""
    T = x.shape[0]
    xt = np.ascontiguousarray(x.T).reshape(NFI, 128, T).transpose(1, 0, 2)
    blocks = [
        xt[:, :, t0:t0 + tw].reshape(128, -1) for (t0, tw) in _blocks(T, first)
    ]
    return np.concatenate(blocks, axis=1).astype(ml_dtypes.bfloat16)


def _swz_w(w):
    """[FO, HID] -> [128, NFI*FO]: w.T reshaped to [128, NFI, FO] flat."""
    wt = np.ascontiguousarray(w.T).reshape(NFI, 128, FO).transpose(1, 0, 2)
    return np.ascontiguousarray(wt.reshape(128, NFI * FO)).astype(ml_dtypes.bfloat16)


def kernel(q, k, v, Wq, bq, Wk, bk, Wv, bv, mask_attn, mask_out):
    q = np.asarray(q, np.float32)
    k = np.asarray(k, np.float32)
    v = np.asarray(v, np.float32)
    Wq = np.asarray(Wq, np.float32)
    Wk = np.asarray(Wk, np.float32)
    Wv = np.asarray(Wv, np.float32)
    bq = np.asarray(bq, np.float32)
    bk = np.asarray(bk, np.float32)
    bv = np.asarray(bv, np.float32)
    mask_attn = np.asarray(mask_attn)
    mask_out = np.asarray(mask_out)

    B, T, _ = q.shape
    idxk = [np.flatnonzero(mask_attn[b]) for b in range(B)]
    idxq = [np.flatnonzero(mask_out[b]) for b in range(B)]
    TK = max(128, -(-max(len(i) for i in idxk) // 128) * 128)
    TQ = max(128, -(-max(len(i) for i in idxq) // 128) * 128)
    NTK = TK // 128

    nc = _build(TQ, TK)

    in_maps = []
    for c in range(N_CORES):
        b, g = c // 2, c % 2
        sl = slice(g * FO, (g + 1) * FO)
        nk, nq = len(idxk[b]), len(idxq[b])
        qc = np.zeros((TQ, HID), np.float32)
        qc[:nq] = q[b][idxq[b]]
        kc = np.zeros((TK, HID), np.float32)
        kc[:nk] = k[b][idxk[b]]
        vc = np.zeros((TK, HID), np.float32)
        vc[:nk] = v[b][idxk[b]]
        biask = np.full(TK, -30000.0, np.float32)
        biask[:nk] = 0.0
        in_maps.append({
            "qT": _swz_act(qc, 256),
            "kT": _swz_act(kc, 256),
            "vT": _swz_act(vc, 128),
            "wqT": _swz_w(Wq[sl]),
            "wkT": _swz_w(Wk[sl]),
            "wvT": _swz_w(Wv[sl]),
            "bq": np.ascontiguousarray(bq[sl].reshape(4, 128).T),
            "bk": np.ascontiguousarray(bk[sl].reshape(4, 128).T),
            "bv": bv[sl].reshape(1, FO).astype(ml_dtypes.bfloat16),
            "biask": np.ascontiguousarray(biask.reshape(NTK, 128).T),
            "ones1": np.ones((1, 512), ml_dtypes.bfloat16),
            "onesv": np.ones((128, NTK * HPC), ml_dtypes.bfloat16),
        })

    trace_dir = os.environ.get("KERNEL_TRACE_DIR")
    if trace_dir:
        res = run_bass_kernel_spmd(
            nc, in_maps, list(range(N_CORES)), trace=True, tmpdir=trace_dir
        )
        print(f"HW exec time: {res.exec_time_ns} ns")
    else:
        res = run_bass_kernel_spmd(nc, in_maps, list(range(N_CORES)))

    out_full = np.zeros((B, T, HID), np.float32)
    for c in range(N_CORES):
        b, g = c // 2, c % 2
        nq = len(idxq[b])
        u = res.results[c]["out"]  # [65, HPC, TQ]
        o = u[:64, :, :nq] / u[64:65, :, :nq]
        o = o.transpose(2, 1, 0).reshape(nq, FO)
        out_full[b, idxq[b], g * FO:(g + 1) * FO] = o
    return out_full
